# revision 38
# baseline (speedup 1.0000x reference)
"""Transformer block (post-LN, BERT-style) on 8 TRN2 NeuronCores, collective-free.

Sharding: 8 cores = 4 batches x 2 query-halves. Core c=(b,j) computes, for
batch b:
  - K/V projections for all 2048 tokens (recomputed per core pair; cheaper
    than any collective at this size),
  - Q projection + attention + output projection for its own 1024 query
    tokens (all 16 heads),
  - LN1, full FFN, LN2 for its 1024 tokens.
Host concatenates the 8 [1024, 1024] output slices. No collectives.

Layouts keep activations transposed ([feature, token]) so every matmul uses
weights in natural layout; x arrives pre-transposed from the host; softmax
row-sums come from a ones-column appended to V; 1/sqrt(hd) folds into the Q
projection epilogue.
"""

import os
import sys
import types
import numpy as np

import concourse.bacc as bacc
import concourse.bass as bass
import concourse.tile as tile
import concourse.mybir as mybir
from concourse.bass_utils import run_bass_kernel_spmd

P = 128
F32 = mybir.dt.float32
BF16 = mybir.dt.bfloat16
FP8 = mybir.dt.float8e4
AF = mybir.ActivationFunctionType
ALU = mybir.AluOpType

NH_CORE = 16  # heads per core (all of them; cores split over batch x seq)


def build_block(nc, *, S, H, NH_core, FF, eps=1e-12, flags=None, prefix=""):
    """Emit the SPMD program for one core. flags: set of optional-input names
    among {mask, bq, bk, bv, bo, b1, b2, ln1_g, ln1_b, ln2_g, ln2_b} that are
    actually present (nonzero / non-one)."""
    flags = flags or set()
    HD = 64
    NH = NH_core               # 16 heads, all on this core
    SQ = S // 2                # query tokens owned by this core
    HT = H // P                # 8 feature subtiles of H
    KT = S // P                # 16 k-token tiles
    KG = 2                     # k-tiles per exp batch (2 PSUM banks/slot)
    NKG = KT // KG
    QC = 512                   # query chunk (tokens per attention sweep)
    NQC = SQ // QC             # 2
    TC = 512                   # token chunk in projections
    HOC = 512                  # H-output chunk
    NHOC = H // HOC
    NFQ = 4                    # stream FFN weights in quarters
    FQ = FF // NFQ
    FTQ = FQ // P
    TT_Q = QC // P             # 4 token tiles per query chunk

    def pn(n):
        return f"{prefix}{n}"

    def param(name, shape, dt=F32):
        return nc.declare_dram_parameter(pn(name), list(shape), dt,
                                         isOutput=False)

    xT = param("xT", [H, S], BF16)
    xqT = param("xqT", [H, SQ], BF16)
    xh = param("xh", [SQ, H])
    wq = param("wq", [H, H], BF16)
    wk = param("wk", [H, H], BF16)
    wv = param("wv", [H, H], BF16)
    wo = param("wo", [H, H], BF16)
    w1 = param("w1", [H, FF], BF16)
    w2 = param("w2", [FF, H], BF16)
    opt = {}
    for name, shape in [("mask", [S]), ("bq", [H]), ("bk", [H]), ("bv", [H]),
                        ("bo", [H]), ("b1", [FF]), ("b2", [H]),
                        ("ln1_g", [H]), ("ln1_b", [H]),
                        ("ln2_g", [H]), ("ln2_b", [H])]:
        if name in flags:
            opt[name] = param(name, shape)
    out_ext = nc.declare_dram_parameter(pn("out"), [SQ, H], F32, isOutput=True)

    with (
        tile.TileContext(nc) as tc,
        tc.tile_pool(name=pn("singles"), bufs=1) as singles,
        tc.tile_pool(name=pn("dram"), bufs=1, space="DRAM") as dram,
    ):
        eps_sb = singles.tile([P, 1], F32)
        nc.vector.memset(eps_sb, eps)
        mask_sb = None
        if "mask" in flags:
            mask_sb = singles.tile([P, KT], F32)
            nc.gpsimd.dma_start(mask_sb, opt["mask"].rearrange("(a p) -> p a", p=P))

        # per-partition bias strips ([P, n//P]: feature f at [f%P, f//P])
        def col_strip(name, n):
            if name not in flags:
                return None
            t = singles.tile([P, n // P], F32, tag=f"strip_{name}")
            nc.gpsimd.dma_start(t, opt[name].rearrange("(a p) -> p a", p=P))
            return t
        bq_sb = col_strip("bq", H)
        bk_sb = col_strip("bk", H)
        b1_sb = col_strip("b1", FF)

        # partition-replicated rows (for free-dim adds)
        def rep_row(name, n):
            if name not in flags:
                return None
            t = singles.tile([P, n], F32, tag=f"rep_{name}")
            src = opt[name][:]
            bcast = bass.AP(tensor=src.tensor, offset=src.offset,
                            ap=[[0, P]] + list(src.ap))
            nc.gpsimd.dma_start(t, bcast)
            return t
        bv_sb = rep_row("bv", H)
        bo_sb = rep_row("bo", H)
        b2_sb = rep_row("b2", H)
        ln1g_sb = rep_row("ln1_g", H)
        ln1b_sb = rep_row("ln1_b", H)
        ln2g_sb = rep_row("ln2_g", H)
        ln2b_sb = rep_row("ln2_b", H)

        ones_sb = singles.tile([P, HD], F32)
        nc.vector.memset(ones_sb, 1.0)
        x1_dram = dram.tile([SQ, H], BF16)

        SG = 512                      # layernorm bn_stats chunk
        NSG = H // SG

        def layernorm_tile(lntp, y_t, out_sl, g_sb, b_sb):
            st6 = lntp.tile([P, NSG, 6], F32, tag="st6")
            for sg in range(NSG):
                nc.vector.bn_stats(st6[:, sg, :], y_t[:, sg * SG:(sg + 1) * SG])
            mv = lntp.tile([P, 2], F32, tag="mv")
            nc.vector.bn_aggr(mv, st6)
            nc.scalar.activation(mv[:, 1:2], mv[:, 1:2], AF.Sqrt, bias=eps_sb)
            nc.vector.reciprocal(mv[:, 1:2], mv[:, 1:2])
            nc.vector.tensor_scalar(out_sl, y_t, mv[:, 0:1], mv[:, 1:2],
                                    ALU.subtract, ALU.mult)
            if g_sb is not None:
                nc.vector.tensor_tensor(out_sl, out_sl, g_sb, ALU.mult)
            if b_sb is not None:
                nc.vector.tensor_tensor(out_sl, out_sl, b_sb, ALU.add)

        ST2 = SQ // P            # 8 token tiles
        x1T_c = [singles.tile([P, HT, QC], BF16, tag=f"x1T{i}",
                               name=f"x1T{i}")
                 for i in range(NQC)]
        with (
            tc.tile_pool(name=pn("wo"), bufs=1) as wop,
            tc.tile_pool(name=pn("stage"), bufs=3) as stagep,
            tc.tile_pool(name=pn("ctxk"), bufs=2) as ctxkp,
            tc.tile_pool(name=pn("ln1"), bufs=2) as ln1p,
            tc.tile_pool(name=pn("wo_ps"), bufs=1, space="PSUM") as wops,
        ):
            wo_sb = wop.tile([P, HT, H], BF16)
            nc.gpsimd.dma_start(wo_sb, wo.rearrange("(a p) h -> p a h", p=P))
            ctxT_per_qc = []
            keep_ctx = tc.tile_pool(name=pn("attn_keep"), bufs=1)
            keep = keep_ctx.__enter__()
            qT = keep.tile([P, HT, SQ], BF16)
            kT = keep.tile([P, HT, S], BF16)
            # fp8 V (+ones col): quantization error lands on the attention
            # output, which is self-normalized and tiny next to the residual.
            # Row stride HD+2 keeps every slice at an even byte offset; the
            # full-tile memset(1.0) provides the ones column contiguously.
            v_sb = keep.tile([P, KT, NH, HD + 2], FP8)
            nc.vector.memset(v_sb, 1.0)

            # ---------------- phase A: projections ------------------------
            with (
                tc.tile_pool(name=pn("qw"), bufs=1) as qwp,
                tc.tile_pool(name=pn("qkv_ps"), bufs=4, space="PSUM") as qps,
            ):
                # Q first: small DMA footprint, warms the PE early.
                xqT_sb = qwp.tile([P, HT, SQ], BF16)
                wq_sb = qwp.tile([P, HT, H], BF16)
                xqTr = xqT.rearrange("(a p) t -> p a t", p=P)
                nc.sync.dma_start(wq_sb, wq.rearrange("(a p) d -> p a d", p=P))
                for tci in range(SQ // TC):
                    t_sl = slice(tci * TC, (tci + 1) * TC)
                    nc.sync.dma_start(xqT_sb[:, :, t_sl], xqTr[:, :, t_sl])
                    for dt in range(HT):
                        ps = qps.tile([P, TC], F32, tag="qk")
                        for ht in range(HT):
                            nc.tensor.matmul(
                                ps, wq_sb[:, ht, dt * P:(dt + 1) * P],
                                xqT_sb[:, ht, t_sl],
                                start=(ht == 0), stop=(ht == HT - 1))
                        d_sl = qT[:, dt, t_sl]
                        if bq_sb is not None:
                            nc.vector.tensor_scalar(
                                d_sl, ps, bq_sb[:, dt:dt + 1], 0.125,
                                ALU.add, ALU.mult)
                        else:
                            nc.vector.tensor_scalar_mul(d_sl, ps, 0.125)

            with (
                tc.tile_pool(name=pn("kvw"), bufs=1) as kvwp,
                tc.tile_pool(name=pn("xtc"), bufs=2) as xtcp,
                tc.tile_pool(name=pn("kv_ps"), bufs=4, space="PSUM") as kvps,
                tc.tile_pool(name=pn("v_ps"), bufs=3, space="PSUM") as vps,
            ):
                wk_sb = kvwp.tile([P, HT, H], BF16)
                wv_sb = kvwp.tile([P, HT, H], BF16)
                # scalar-queue DMAs run in parallel with the sync-queue loads
                # above, so kT can start right after the Q matmuls. wv goes
                # after the first x chunk: V matmuls only start post-kT.
                nc.scalar.dma_start(wk_sb, wk.rearrange("(a p) d -> p a d", p=P))
                xTr = xT.rearrange("(a p) t -> p a t", p=P)
                for tci in range(S // TC):
                    t_sl = slice(tci * TC, (tci + 1) * TC)
                    xT_c = xtcp.tile([P, HT, TC], BF16, tag="xc")
                    nc.scalar.dma_start(xT_c, xTr[:, :, t_sl])
                    if tci == 0:
                        nc.scalar.dma_start(
                            wv_sb, wv.rearrange("(a p) d -> p a d", p=P))
                    for dt in range(HT):
                        ps = kvps.tile([P, TC], F32, tag="k")
                        for ht in range(HT):
                            nc.tensor.matmul(
                                ps, wk_sb[:, ht, dt * P:(dt + 1) * P],
                                xT_c[:, ht, :],
                                start=(ht == 0), stop=(ht == HT - 1))
                        d_sl = kT[:, dt, t_sl]
                        if bk_sb is not None:
                            nc.vector.tensor_scalar(
                                d_sl, ps, bk_sb[:, dt:dt + 1], 1.0,
                                ALU.add, ALU.mult)
                        else:
                            nc.vector.tensor_copy(d_sl, ps)
                    # V for this token chunk: token-major, ones col at HD
                    for lt in range(TC // P):
                        tt = tci * (TC // P) + lt
                        for dh in range(2):
                            ps = vps.tile([P, HOC], F32, tag="v")
                            for ht in range(HT):
                                nc.tensor.matmul(
                                    ps, xT_c[:, ht, lt * P:(lt + 1) * P],
                                    wv_sb[:, ht, dh * HOC:(dh + 1) * HOC],
                                    start=(ht == 0), stop=(ht == HT - 1))
                            if bv_sb is not None:
                                nc.vector.tensor_tensor(
                                    ps, ps, bv_sb[:, dh * HOC:(dh + 1) * HOC],
                                    ALU.add)
                            nc.vector.tensor_copy(
                                v_sb[:, tt, dh * 8:(dh + 1) * 8, 0:HD],
                                ps.rearrange("p (nh hd) -> p nh hd", hd=HD))

            # ---------------- phase B: attention -------------------------
            with (
                tc.tile_pool(name=pn("probs"), bufs=2) as probsp,
                tc.tile_pool(name=pn("sc_ps"), bufs=2, space="PSUM") as scp,
                tc.tile_pool(name=pn("ctx_ps"), bufs=3, space="PSUM") as ctxp,
            ):
                for qc in range(NQC):
                    q_sl = slice(qc * QC, (qc + 1) * QC)
                    ctxT = ctxkp.tile([P, HT, QC], BF16, tag=f"ctxT{qc}")
                    ctxT_per_qc.append(ctxT)
                    # Head-PAIR pipeline. Scores for heads (2hp, 2hp+1) sit at
                    # complementary partition halves, so their K=64 matmuls
                    # row-pack into the full PE array (HAM counts half-array
                    # matmuls as idle — packing keeps the clock at 2.4GHz).
                    # The previous pair's ctx matmuls interleave per k-tile so
                    # the PE never idles while ACT works through the exps.
                    prev = None
                    for hp in range(NH // 2 + 1):
                        if hp < NH // 2:
                            probs = probsp.tile([P, KT, 2, QC], FP8,
                                                tag="probs")
                        pce = pco = None
                        for kt in range(KT):
                            if hp < NH // 2:
                                ps_s = scp.tile([P, 2, QC], F32, tag="sc")
                                for par in range(2):
                                    hs = par * HD
                                    nc.tensor.matmul(
                                        ps_s[:, par, :],
                                        kT[hs:hs + HD, hp, kt * P:(kt + 1) * P],
                                        qT[hs:hs + HD, hp, q_sl],
                                        start=True, stop=True)
                                if mask_sb is not None:
                                    mvw = mask_sb[:, kt:kt + 1, None]
                                    nc.vector.tensor_tensor(
                                        ps_s, ps_s,
                                        mvw.to_broadcast((P, 2, QC)), ALU.add)
                                nc.scalar.activation(
                                    probs[:, kt, :, :], ps_s, AF.Exp)
                            if prev is not None:
                                php, pprobs = prev
                                if kt == 0:
                                    pce = ctxp.tile([P, QC], F32, tag="ctx")
                                    pco = ctxp.tile([P, QC], F32, tag="ctx")
                                nc.tensor.matmul(
                                    pce[0:HD + 1, :],
                                    v_sb[:, kt, 2 * php, 0:HD + 1],
                                    pprobs[:, kt, 0, :],
                                    start=(kt == 0), stop=(kt == KT - 1))
                                nc.tensor.matmul(
                                    pco[0:HD + 1, :],
                                    v_sb[:, kt, 2 * php + 1, 0:HD + 1],
                                    pprobs[:, kt, 1, :],
                                    start=(kt == 0), stop=(kt == KT - 1))
                        if prev is not None:
                            php, pprobs = prev
                            # softmax normalize: 1/rowsum broadcast over the
                            # 64 hd partitions via a K=1 fp32 matmul, fused
                            # into the PSUM->SBUF copy.
                            for par, ps_pc in ((0, pce), (1, pco)):
                                phs = par * HD
                                cs = stagep.tile([P, QC], BF16, tag="cs")
                                rr = stagep.tile([P, QC], F32, tag="rr")
                                nc.vector.reciprocal(
                                    rr[HD:HD + 1, :], ps_pc[HD:HD + 1, :])
                                rb = wops.tile([P, QC], F32, tag="rbwo")
                                nc.tensor.matmul(rb[0:HD, :],
                                                 ones_sb[HD:HD + 1, :],
                                                 rr[HD:HD + 1, :],
                                                 start=True, stop=True)
                                nc.vector.tensor_copy(cs[0:HD, :],
                                                      ps_pc[0:HD, :])
                                nc.vector.tensor_tensor(
                                    cs[0:HD, :], cs[0:HD, :],
                                    rb[0:HD, :], ALU.mult)
                                nc.sync.dma_start(ctxT[phs:phs + HD, php, :],
                                                  cs[0:HD, :])
                        if hp < NH // 2:
                            prev = (hp, probs)
                        else:
                            prev = None

            # close the K/Q/V pool before Wo+LN1 so the FFN's first tiles can
            # allocate (and its matmuls fill the transition gap).
            keep_ctx.__exit__(None, None, None)

            # ---------------- Wo + residual + LN1 ------------------------
            for qc in range(NQC):
                ctxT = ctxT_per_qc[qc]
                for tt in range(TT_Q):
                    tok0 = qc * QC + tt * P
                    xh_t = ln1p.tile([P, H], F32, tag="xh")
                    nc.sync.dma_start(xh_t, xh[tok0:tok0 + P, :])
                    y_t = ln1p.tile([P, H], F32, tag="y")
                    for hoc in range(NHOC):
                        o_sl = slice(hoc * HOC, (hoc + 1) * HOC)
                        ps_a = wops.tile([P, HOC], F32, tag="rbwo")
                        for st in range(HT):
                            nc.tensor.matmul(
                                ps_a,
                                ctxT[:, st, tt * P:(tt + 1) * P],
                                wo_sb[:, st, o_sl],
                                start=(st == 0), stop=(st == HT - 1))
                        nc.vector.tensor_tensor(y_t[:, o_sl], ps_a,
                                                xh_t[:, o_sl], ALU.add)
                    if bo_sb is not None:
                        nc.vector.tensor_tensor(y_t, y_t, bo_sb, ALU.add)
                    x1b_t = ln1p.tile([P, H], BF16, tag="x1b")
                    layernorm_tile(ln1p, y_t, x1b_t, ln1g_sb, ln1b_sb)
                    nc.sync.dma_start(x1_dram[tok0:tok0 + P, :], x1b_t)
                    nc.sync.dma_start_transpose(
                        x1T_c[qc][:, :, tt * P:(tt + 1) * P], x1b_t)

        # ---------------- phase C: FFN + LN2 ------------------------------
        with (
            tc.tile_pool(name=pn("ffn_w"), bufs=2) as fwp,
            tc.tile_pool(name=pn("gt"), bufs=2) as gtp,
            tc.tile_pool(name=pn("out2"), bufs=1) as out2p,
            tc.tile_pool(name=pn("ln2"), bufs=2) as ln2p,
            tc.tile_pool(name=pn("h_ps"), bufs=4, space="PSUM") as hps,
            tc.tile_pool(name=pn("o_ps"), bufs=4, space="PSUM") as ops,
        ):
            out2 = out2p.tile([P, ST2, H], F32)
            for fq in range(NFQ):
                f_sl = slice(fq * FQ, (fq + 1) * FQ)
                # fq 0/1 on the (idle) gpsimd queue so the first FFN weights
                # land while attention still owns the sync queue.
                dma_eng = nc.gpsimd if fq < 2 else nc.sync
                w1q = fwp.tile([P, HT, FQ], BF16, tag="w1q")
                dma_eng.dma_start(
                    w1q, w1[:, f_sl].rearrange("(a p) f -> p a f", p=P))
                w2q = fwp.tile([P, FTQ, H], BF16, tag="w2q")
                dma_eng.dma_start(
                    w2q, w2[f_sl, :].rearrange("(a p) h -> p a h", p=P))
                gt = gtp.tile([P, FTQ, SQ], BF16, tag="gt")
                for ft in range(FTQ):
                    for tci in range(SQ // TC):
                        ps = hps.tile([P, TC], F32, tag="h")
                        for ht in range(HT):
                            nc.tensor.matmul(
                                ps, w1q[:, ht, ft * P:(ft + 1) * P],
                                x1T_c[tci][:, ht, :],
                                start=(ht == 0), stop=(ht == HT - 1))
                        bias = (b1_sb[:, fq * FTQ + ft:fq * FTQ + ft + 1]
                                if b1_sb is not None else 0.0)
                        nc.scalar.activation(
                            gt[:, ft, tci * TC:(tci + 1) * TC], ps,
                            AF.Gelu_apprx_tanh, bias=bias)
                last = fq == NFQ - 1
                for tt in range(ST2):
                    if last:
                        x1l2 = ln2p.tile([P, H], BF16, tag="x1l2")
                        nc.scalar.dma_start(x1l2,
                                            x1_dram[tt * P:(tt + 1) * P, :])
                        y_t = ln2p.tile([P, H], F32, tag="y2")
                    for hoc in range(NHOC):
                        o_sl = slice(hoc * HOC, (hoc + 1) * HOC)
                        ps2 = ops.tile([P, HOC], F32, tag="o")
                        for ft in range(FTQ):
                            nc.tensor.matmul(
                                ps2, gt[:, ft, tt * P:(tt + 1) * P],
                                w2q[:, ft, o_sl],
                                start=(ft == 0), stop=(ft == FTQ - 1))
                        if fq == 0:
                            nc.vector.tensor_copy(out2[:, tt, o_sl], ps2)
                        elif not last:
                            nc.vector.tensor_tensor(
                                out2[:, tt, o_sl], out2[:, tt, o_sl],
                                ps2, ALU.add)
                        else:
                            # fold the final quarter + LN2 residual per tile
                            nc.vector.tensor_tensor(
                                y_t[:, o_sl], out2[:, tt, o_sl], ps2, ALU.add)
                    if last:
                        nc.vector.tensor_tensor(y_t, y_t, x1l2, ALU.add)
                        if b2_sb is not None:
                            nc.vector.tensor_tensor(y_t, y_t, b2_sb, ALU.add)
                        o_t = ln2p.tile([P, H], F32, tag="o")
                        layernorm_tile(ln2p, y_t, o_t, ln2g_sb, ln2b_sb)
                        nc.sync.dma_start(out_ext[tt * P:(tt + 1) * P, :], o_t)


# ---------------------------------------------------------------------------
# host side
# ---------------------------------------------------------------------------

def _nonzero(a):
    return bool(np.any(np.asarray(a) != 0))


def compute_flags(inputs):
    flags = set()
    if _nonzero(inputs["attention_mask"]):
        flags.add("mask")
    for name in ["bq", "bk", "bv", "bo", "b1", "b2", "ln1_b", "ln2_b"]:
        if _nonzero(inputs[name]):
            flags.add(name)
    for name in ["ln1_g", "ln2_g"]:
        if bool(np.any(np.asarray(inputs[name]) != 1)):
            flags.add(name)
    return flags


def make_in_maps(S, H, FF, inputs, flags):
    """Shard full inputs into 8 per-core input maps (big tensors as bf16)."""
    import ml_dtypes
    bf16 = ml_dtypes.bfloat16
    SQ = S // 2
    x = np.asarray(inputs["x"], np.float32)       # [4, S, H]
    shared = {
        "wq": np.ascontiguousarray(np.asarray(inputs["Wq"], np.float32)).astype(bf16),
        "wk": np.ascontiguousarray(np.asarray(inputs["Wk"], np.float32)).astype(bf16),
        "wv": np.ascontiguousarray(np.asarray(inputs["Wv"], np.float32)).astype(bf16),
        "wo": np.ascontiguousarray(np.asarray(inputs["Wo"], np.float32)).astype(bf16),
        "w1": np.ascontiguousarray(np.asarray(inputs["W1"], np.float32)).astype(bf16),
        "w2": np.ascontiguousarray(np.asarray(inputs["W2"], np.float32)).astype(bf16),
    }
    for name in ["bq", "bk", "bv", "bo", "b1", "b2",
                 "ln1_g", "ln1_b", "ln2_g", "ln2_b"]:
        if name in flags:
            src = {"bq": "bq", "bk": "bk", "bv": "bv", "bo": "bo",
                   "b1": "b1", "b2": "b2", "ln1_g": "ln1_g", "ln1_b": "ln1_b",
                   "ln2_g": "ln2_g", "ln2_b": "ln2_b"}[name]
            shared[name] = np.ascontiguousarray(
                np.asarray(inputs[src], np.float32))
    xT_by_batch = [np.ascontiguousarray(x[b].T).astype(bf16) for b in range(4)]
    maps = []
    for c in range(8):
        b, j = divmod(c, 2)
        xTb = xT_by_batch[b]
        m = dict(shared)
        m["xT"] = xTb
        m["xqT"] = np.ascontiguousarray(xTb[:, j * SQ:(j + 1) * SQ])
        m["xh"] = np.ascontiguousarray(x[b, j * SQ:(j + 1) * SQ])
        if "mask" in flags:
            m["mask"] = np.ascontiguousarray(
                np.asarray(inputs["attention_mask"], np.float32)[b, 0, 0, :])
        maps.append(m)
    return maps


LAST_EXEC_NS = None
LAST_RESULTS = None


def _install_ntff_hook():
    """Register the NTFF profiling hook (missing antenv.axon_hooks shim)."""
    if "antenv.axon_hooks" in sys.modules:
        return
    try:
        import antenv  # noqa: F401
        mod = types.ModuleType("antenv.axon_hooks")
        hook = [None]
        mod.set_axon_ntff_profile_hook = lambda h: hook.__setitem__(0, h)
        mod.get_axon_ntff_profile_hook = lambda: hook[0]
        sys.modules["antenv.axon_hooks"] = mod
        from trn_agent_boot.trn_boot import _ntff_profile_via_ctypes
        mod.set_axon_ntff_profile_hook(
            _ntff_profile_via_ctypes("/opt/axon/libaxon_pjrt.so"))
    except Exception:
        sys.modules.pop("antenv.axon_hooks", None)


def run_block(S, H, FF, inputs, trace=False):
    """Build, compile, run on 8 cores; returns [B, S, H] output."""
    global LAST_EXEC_NS, LAST_RESULTS
    flags = compute_flags(inputs)
    nc = bacc.Bacc("TRN2", target_bir_lowering=False, debug=True)
    build_block(nc, S=S, H=H, NH_core=NH_CORE, FF=FF, flags=flags)
    nc.compile()
    in_maps = make_in_maps(S, H, FF, inputs, flags)
    if trace:
        _install_ntff_hook()
    res = run_bass_kernel_spmd(
        nc, in_maps, core_ids=list(range(8)), trace=trace,
        trace_cores=[0] if trace else None)
    LAST_EXEC_NS = res.exec_time_ns
    LAST_RESULTS = res
    SQ = S // 2
    B = 4
    out = np.empty((B, S, H), np.float32)
    for c in range(8):
        b, j = divmod(c, 2)
        out[b, j * SQ:(j + 1) * SQ] = res.results[c]["out"]
    return out


def kernel(x, attention_mask, Wq, bq, Wk, bk, Wv, bv, Wo, bo,
           ln1_g, ln1_b, W1, b1, W2, b2, ln2_g, ln2_b):
    inputs = dict(x=x, attention_mask=attention_mask, Wq=Wq, bq=bq, Wk=Wk,
                  bk=bk, Wv=Wv, bv=bv, Wo=Wo, bo=bo, ln1_g=ln1_g,
                  ln1_b=ln1_b, W1=W1, b1=b1, W2=W2, b2=b2, ln2_g=ln2_g,
                  ln2_b=ln2_b)
    trace = bool(int(os.environ.get("BLOCK_TRACE", "0")))
    return run_block(2048, 1024, 4096, inputs, trace=trace)


# revision 39
# speedup vs baseline: 1.0096x; 1.0096x over previous
"""Transformer block (post-LN, BERT-style) on 8 TRN2 NeuronCores, collective-free.

Sharding: 8 cores = 4 batches x 2 query-halves. Core c=(b,j) computes, for
batch b:
  - K/V projections for all 2048 tokens (recomputed per core pair; cheaper
    than any collective at this size),
  - Q projection + attention + output projection for its own 1024 query
    tokens (all 16 heads),
  - LN1, full FFN, LN2 for its 1024 tokens.
Host concatenates the 8 [1024, 1024] output slices. No collectives.

Layouts keep activations transposed ([feature, token]) so every matmul uses
weights in natural layout; x arrives pre-transposed from the host; softmax
row-sums come from a ones-column appended to V; 1/sqrt(hd) folds into the Q
projection epilogue.
"""

import os
import sys
import types
import numpy as np

import concourse.bacc as bacc
import concourse.bass as bass
import concourse.tile as tile
import concourse.mybir as mybir
from concourse.bass_utils import run_bass_kernel_spmd

P = 128
F32 = mybir.dt.float32
BF16 = mybir.dt.bfloat16
FP8 = mybir.dt.float8e4
AF = mybir.ActivationFunctionType
ALU = mybir.AluOpType

NH_CORE = 16  # heads per core (all of them; cores split over batch x seq)


def build_block(nc, *, S, H, NH_core, FF, eps=1e-12, flags=None, prefix=""):
    """Emit the SPMD program for one core. flags: set of optional-input names
    among {mask, bq, bk, bv, bo, b1, b2, ln1_g, ln1_b, ln2_g, ln2_b} that are
    actually present (nonzero / non-one)."""
    flags = flags or set()
    HD = 64
    NH = NH_core               # 16 heads, all on this core
    SQ = S // 2                # query tokens owned by this core
    HT = H // P                # 8 feature subtiles of H
    KT = S // P                # 16 k-token tiles
    KG = 2                     # k-tiles per exp batch (2 PSUM banks/slot)
    NKG = KT // KG
    QC = 512                   # query chunk (tokens per attention sweep)
    NQC = SQ // QC             # 2
    TC = 512                   # token chunk in projections
    HOC = 512                  # H-output chunk
    NHOC = H // HOC
    NFQ = 4                    # stream FFN weights in quarters
    FQ = FF // NFQ
    FTQ = FQ // P
    TT_Q = QC // P             # 4 token tiles per query chunk

    def pn(n):
        return f"{prefix}{n}"

    def param(name, shape, dt=F32):
        return nc.declare_dram_parameter(pn(name), list(shape), dt,
                                         isOutput=False)

    xT = param("xT", [H, S], BF16)
    xqT = param("xqT", [H, SQ], BF16)
    xh = param("xh", [SQ, H])
    wq = param("wq", [H, H], BF16)
    wk = param("wk", [H, H], BF16)
    wv = param("wv", [H, H], BF16)
    wo = param("wo", [H, H], BF16)
    w1 = param("w1", [H, FF], BF16)
    w2 = param("w2", [FF, H], BF16)
    opt = {}
    for name, shape in [("mask", [S]), ("bq", [H]), ("bk", [H]), ("bv", [H]),
                        ("bo", [H]), ("b1", [FF]), ("b2", [H]),
                        ("ln1_g", [H]), ("ln1_b", [H]),
                        ("ln2_g", [H]), ("ln2_b", [H])]:
        if name in flags:
            opt[name] = param(name, shape)
    out_ext = nc.declare_dram_parameter(pn("out"), [SQ, H], F32, isOutput=True)

    with (
        tile.TileContext(nc) as tc,
        tc.tile_pool(name=pn("singles"), bufs=1) as singles,
        tc.tile_pool(name=pn("dram"), bufs=1, space="DRAM") as dram,
    ):
        eps_sb = singles.tile([P, 1], F32)
        nc.vector.memset(eps_sb, eps)
        mask_sb = None
        if "mask" in flags:
            mask_sb = singles.tile([P, KT], F32)
            nc.gpsimd.dma_start(mask_sb, opt["mask"].rearrange("(a p) -> p a", p=P))

        # per-partition bias strips ([P, n//P]: feature f at [f%P, f//P])
        def col_strip(name, n):
            if name not in flags:
                return None
            t = singles.tile([P, n // P], F32, tag=f"strip_{name}")
            nc.gpsimd.dma_start(t, opt[name].rearrange("(a p) -> p a", p=P))
            return t
        bq_sb = col_strip("bq", H)
        bk_sb = col_strip("bk", H)
        b1_sb = col_strip("b1", FF)

        # partition-replicated rows (for free-dim adds)
        def rep_row(name, n):
            if name not in flags:
                return None
            t = singles.tile([P, n], F32, tag=f"rep_{name}")
            src = opt[name][:]
            bcast = bass.AP(tensor=src.tensor, offset=src.offset,
                            ap=[[0, P]] + list(src.ap))
            nc.gpsimd.dma_start(t, bcast)
            return t
        bv_sb = rep_row("bv", H)
        bo_sb = rep_row("bo", H)
        b2_sb = rep_row("b2", H)
        ln1g_sb = rep_row("ln1_g", H)
        ln1b_sb = rep_row("ln1_b", H)
        ln2g_sb = rep_row("ln2_g", H)
        ln2b_sb = rep_row("ln2_b", H)

        ones_sb = singles.tile([P, HD], F32)
        nc.vector.memset(ones_sb, 1.0)
        x1_dram = dram.tile([SQ, H], BF16)

        SG = 512                      # layernorm bn_stats chunk
        NSG = H // SG

        def layernorm_tile(lntp, y_t, out_sl, g_sb, b_sb):
            st6 = lntp.tile([P, NSG, 6], F32, tag="st6")
            for sg in range(NSG):
                nc.vector.bn_stats(st6[:, sg, :], y_t[:, sg * SG:(sg + 1) * SG])
            mv = lntp.tile([P, 2], F32, tag="mv")
            nc.vector.bn_aggr(mv, st6)
            nc.scalar.activation(mv[:, 1:2], mv[:, 1:2], AF.Sqrt, bias=eps_sb)
            nc.vector.reciprocal(mv[:, 1:2], mv[:, 1:2])
            nc.vector.tensor_scalar(out_sl, y_t, mv[:, 0:1], mv[:, 1:2],
                                    ALU.subtract, ALU.mult)
            if g_sb is not None:
                nc.vector.tensor_tensor(out_sl, out_sl, g_sb, ALU.mult)
            if b_sb is not None:
                nc.vector.tensor_tensor(out_sl, out_sl, b_sb, ALU.add)

        ST2 = SQ // P            # 8 token tiles
        x1T_c = [singles.tile([P, HT, QC], BF16, tag=f"x1T{i}",
                               name=f"x1T{i}")
                 for i in range(NQC)]
        with (
            tc.tile_pool(name=pn("wo"), bufs=1) as wop,
            tc.tile_pool(name=pn("stage"), bufs=3) as stagep,
            tc.tile_pool(name=pn("ctxk"), bufs=2) as ctxkp,
            tc.tile_pool(name=pn("ln1"), bufs=2) as ln1p,
            tc.tile_pool(name=pn("wo_ps"), bufs=1, space="PSUM") as wops,
        ):
            wo_sb = wop.tile([P, HT, H], BF16)
            nc.gpsimd.dma_start(wo_sb, wo.rearrange("(a p) h -> p a h", p=P))
            ctxT_per_qc = []
            keep_ctx = tc.tile_pool(name=pn("attn_keep"), bufs=1)
            keep = keep_ctx.__enter__()
            qT = keep.tile([P, HT, SQ], BF16)
            kT = keep.tile([P, HT, S], BF16)
            # fp8 V (+ones col): quantization error lands on the attention
            # output, which is self-normalized and tiny next to the residual.
            # Row stride HD+2 keeps every slice at an even byte offset; the
            # full-tile memset(1.0) provides the ones column contiguously.
            v_sb = keep.tile([P, KT, NH, HD + 2], FP8)
            nc.vector.memset(v_sb, 1.0)

            # ---------------- phase A: projections ------------------------
            with (
                tc.tile_pool(name=pn("qw"), bufs=1) as qwp,
                tc.tile_pool(name=pn("qkv_ps"), bufs=4, space="PSUM") as qps,
            ):
                # Q first: small DMA footprint, warms the PE early.
                xqT_sb = qwp.tile([P, HT, SQ], BF16)
                wq_sb = qwp.tile([P, HT, H], BF16)
                xqTr = xqT.rearrange("(a p) t -> p a t", p=P)
                nc.sync.dma_start(wq_sb, wq.rearrange("(a p) d -> p a d", p=P))
                for tci in range(SQ // TC):
                    t_sl = slice(tci * TC, (tci + 1) * TC)
                    nc.sync.dma_start(xqT_sb[:, :, t_sl], xqTr[:, :, t_sl])
                    for dt in range(HT):
                        ps = qps.tile([P, TC], F32, tag="qk")
                        for ht in range(HT):
                            nc.tensor.matmul(
                                ps, wq_sb[:, ht, dt * P:(dt + 1) * P],
                                xqT_sb[:, ht, t_sl],
                                start=(ht == 0), stop=(ht == HT - 1))
                        d_sl = qT[:, dt, t_sl]
                        if bq_sb is not None:
                            nc.vector.tensor_scalar(
                                d_sl, ps, bq_sb[:, dt:dt + 1], 0.125,
                                ALU.add, ALU.mult)
                        else:
                            nc.vector.tensor_scalar_mul(d_sl, ps, 0.125)

            with (
                tc.tile_pool(name=pn("kvw"), bufs=1) as kvwp,
                tc.tile_pool(name=pn("xtc"), bufs=2) as xtcp,
                tc.tile_pool(name=pn("kv_ps"), bufs=4, space="PSUM") as kvps,
                tc.tile_pool(name=pn("v_ps"), bufs=3, space="PSUM") as vps,
            ):
                wk_sb = kvwp.tile([P, HT, H], BF16)
                wv_sb = kvwp.tile([P, HT, H], BF16)
                # scalar-queue DMAs run in parallel with the sync-queue loads
                # above, so kT can start right after the Q matmuls. wv goes
                # after the first x chunk: V matmuls only start post-kT.
                nc.scalar.dma_start(wk_sb, wk.rearrange("(a p) d -> p a d", p=P))
                xTr = xT.rearrange("(a p) t -> p a t", p=P)
                for tci in range(S // TC):
                    t_sl = slice(tci * TC, (tci + 1) * TC)
                    xT_c = xtcp.tile([P, HT, TC], BF16, tag="xc")
                    nc.scalar.dma_start(xT_c, xTr[:, :, t_sl])
                    if tci == 0:
                        nc.scalar.dma_start(
                            wv_sb, wv.rearrange("(a p) d -> p a d", p=P))
                    for dt in range(HT):
                        ps = kvps.tile([P, TC], F32, tag="k")
                        for ht in range(HT):
                            nc.tensor.matmul(
                                ps, wk_sb[:, ht, dt * P:(dt + 1) * P],
                                xT_c[:, ht, :],
                                start=(ht == 0), stop=(ht == HT - 1))
                        d_sl = kT[:, dt, t_sl]
                        if bk_sb is not None:
                            nc.vector.tensor_scalar(
                                d_sl, ps, bk_sb[:, dt:dt + 1], 1.0,
                                ALU.add, ALU.mult)
                        else:
                            nc.vector.tensor_copy(d_sl, ps)
                    # V for this token chunk: token-major, ones col at HD
                    for lt in range(TC // P):
                        tt = tci * (TC // P) + lt
                        for dh in range(2):
                            ps = vps.tile([P, HOC], F32, tag="v")
                            for ht in range(HT):
                                nc.tensor.matmul(
                                    ps, xT_c[:, ht, lt * P:(lt + 1) * P],
                                    wv_sb[:, ht, dh * HOC:(dh + 1) * HOC],
                                    start=(ht == 0), stop=(ht == HT - 1))
                            if bv_sb is not None:
                                nc.vector.tensor_tensor(
                                    ps, ps, bv_sb[:, dh * HOC:(dh + 1) * HOC],
                                    ALU.add)
                            nc.vector.tensor_copy(
                                v_sb[:, tt, dh * 8:(dh + 1) * 8, 0:HD],
                                ps.rearrange("p (nh hd) -> p nh hd", hd=HD))

            # ---------------- phase B: attention -------------------------
            with (
                tc.tile_pool(name=pn("probs"), bufs=2) as probsp,
                tc.tile_pool(name=pn("sc_ps"), bufs=2, space="PSUM") as scp,
                tc.tile_pool(name=pn("ctx_ps"), bufs=3, space="PSUM") as ctxp,
            ):
                for qc in range(NQC):
                    q_sl = slice(qc * QC, (qc + 1) * QC)
                    ctxT = ctxkp.tile([P, HT, QC], BF16, tag=f"ctxT{qc}")
                    ctxT_per_qc.append(ctxT)
                    # Head-PAIR pipeline. Scores for heads (2hp, 2hp+1) sit at
                    # complementary partition halves, so their K=64 matmuls
                    # row-pack into the full PE array (HAM counts half-array
                    # matmuls as idle — packing keeps the clock at 2.4GHz).
                    # The previous pair's ctx matmuls interleave per k-tile so
                    # the PE never idles while ACT works through the exps.
                    prev = None
                    for hp in range(NH // 2 + 1):
                        if hp < NH // 2:
                            probs = probsp.tile([P, KT, 2, QC], FP8,
                                                tag="probs")
                        pce = pco = None
                        for kt in range(KT):
                            if hp < NH // 2:
                                ps_s = scp.tile([P, 2, QC], F32, tag="sc")
                                for par in range(2):
                                    hs = par * HD
                                    nc.tensor.matmul(
                                        ps_s[:, par, :],
                                        kT[hs:hs + HD, hp, kt * P:(kt + 1) * P],
                                        qT[hs:hs + HD, hp, q_sl],
                                        start=True, stop=True)
                                if mask_sb is not None:
                                    mvw = mask_sb[:, kt:kt + 1, None]
                                    nc.vector.tensor_tensor(
                                        ps_s, ps_s,
                                        mvw.to_broadcast((P, 2, QC)), ALU.add)
                                nc.scalar.activation(
                                    probs[:, kt, :, :], ps_s, AF.Exp)
                            if prev is not None:
                                php, pprobs = prev
                                if kt == 0:
                                    pce = ctxp.tile([P, QC], F32, tag="ctx")
                                    pco = ctxp.tile([P, QC], F32, tag="ctx")
                                nc.tensor.matmul(
                                    pce[0:HD + 1, :],
                                    v_sb[:, kt, 2 * php, 0:HD + 1],
                                    pprobs[:, kt, 0, :],
                                    start=(kt == 0), stop=(kt == KT - 1))
                                nc.tensor.matmul(
                                    pco[0:HD + 1, :],
                                    v_sb[:, kt, 2 * php + 1, 0:HD + 1],
                                    pprobs[:, kt, 1, :],
                                    start=(kt == 0), stop=(kt == KT - 1))
                        if prev is not None:
                            php, pprobs = prev
                            # softmax normalize: 1/rowsum broadcast over the
                            # 64 hd partitions via a K=1 fp32 matmul, fused
                            # into the PSUM->SBUF copy.
                            for par, ps_pc in ((0, pce), (1, pco)):
                                phs = par * HD
                                cs = stagep.tile([P, QC], BF16, tag="cs")
                                rr = stagep.tile([P, QC], F32, tag="rr")
                                nc.vector.reciprocal(
                                    rr[HD:HD + 1, :], ps_pc[HD:HD + 1, :])
                                rb = wops.tile([P, QC], F32, tag="rbwo")
                                nc.tensor.matmul(rb[0:HD, :],
                                                 ones_sb[HD:HD + 1, :],
                                                 rr[HD:HD + 1, :],
                                                 start=True, stop=True)
                                nc.vector.tensor_copy(cs[0:HD, :],
                                                      ps_pc[0:HD, :])
                                nc.vector.tensor_tensor(
                                    cs[0:HD, :], cs[0:HD, :],
                                    rb[0:HD, :], ALU.mult)
                                nc.sync.dma_start(ctxT[phs:phs + HD, php, :],
                                                  cs[0:HD, :])
                        if hp < NH // 2:
                            prev = (hp, probs)
                        else:
                            prev = None

            # close the K/Q/V pool before Wo+LN1 so the FFN's first tiles can
            # allocate (and its matmuls fill the transition gap).
            keep_ctx.__exit__(None, None, None)

            # ---------------- Wo + residual + LN1 ------------------------
            # Dedicated PSUM pool (opened after the attention pools closed)
            # so Wo(qc0) doesn't queue behind all the attention rb slots.
            woq_ctx = tc.tile_pool(name=pn("woq_ps"), bufs=2, space="PSUM")
            woqps = woq_ctx.__enter__()
            for qc in range(NQC):
                ctxT = ctxT_per_qc[qc]
                for tt in range(TT_Q):
                    tok0 = qc * QC + tt * P
                    xh_t = ln1p.tile([P, H], F32, tag="xh")
                    nc.sync.dma_start(xh_t, xh[tok0:tok0 + P, :])
                    y_t = ln1p.tile([P, H], F32, tag="y")
                    for hoc in range(NHOC):
                        o_sl = slice(hoc * HOC, (hoc + 1) * HOC)
                        ps_a = woqps.tile([P, HOC], F32, tag="woq")
                        for st in range(HT):
                            nc.tensor.matmul(
                                ps_a,
                                ctxT[:, st, tt * P:(tt + 1) * P],
                                wo_sb[:, st, o_sl],
                                start=(st == 0), stop=(st == HT - 1))
                        nc.vector.tensor_tensor(y_t[:, o_sl], ps_a,
                                                xh_t[:, o_sl], ALU.add)
                    if bo_sb is not None:
                        nc.vector.tensor_tensor(y_t, y_t, bo_sb, ALU.add)
                    x1b_t = ln1p.tile([P, H], BF16, tag="x1b")
                    layernorm_tile(ln1p, y_t, x1b_t, ln1g_sb, ln1b_sb)
                    nc.sync.dma_start(x1_dram[tok0:tok0 + P, :], x1b_t)
                    nc.sync.dma_start_transpose(
                        x1T_c[qc][:, :, tt * P:(tt + 1) * P], x1b_t)
            woq_ctx.__exit__(None, None, None)

        # ---------------- phase C: FFN + LN2 ------------------------------
        with (
            tc.tile_pool(name=pn("ffn_w"), bufs=2) as fwp,
            tc.tile_pool(name=pn("gt"), bufs=2) as gtp,
            tc.tile_pool(name=pn("out2"), bufs=1) as out2p,
            tc.tile_pool(name=pn("ln2"), bufs=2) as ln2p,
            tc.tile_pool(name=pn("h_ps"), bufs=4, space="PSUM") as hps,
            tc.tile_pool(name=pn("o_ps"), bufs=4, space="PSUM") as ops,
        ):
            out2 = out2p.tile([P, ST2, H], F32)
            for fq in range(NFQ):
                f_sl = slice(fq * FQ, (fq + 1) * FQ)
                # fq 0/1 on the (idle) gpsimd queue so the first FFN weights
                # land while attention still owns the sync queue.
                dma_eng = nc.gpsimd if fq < 2 else nc.sync
                w1q = fwp.tile([P, HT, FQ], BF16, tag="w1q")
                dma_eng.dma_start(
                    w1q, w1[:, f_sl].rearrange("(a p) f -> p a f", p=P))
                w2q = fwp.tile([P, FTQ, H], BF16, tag="w2q")
                dma_eng.dma_start(
                    w2q, w2[f_sl, :].rearrange("(a p) h -> p a h", p=P))
                gt = gtp.tile([P, FTQ, SQ], BF16, tag="gt")
                for ft in range(FTQ):
                    for tci in range(SQ // TC):
                        ps = hps.tile([P, TC], F32, tag="h")
                        for ht in range(HT):
                            nc.tensor.matmul(
                                ps, w1q[:, ht, ft * P:(ft + 1) * P],
                                x1T_c[tci][:, ht, :],
                                start=(ht == 0), stop=(ht == HT - 1))
                        bias = (b1_sb[:, fq * FTQ + ft:fq * FTQ + ft + 1]
                                if b1_sb is not None else 0.0)
                        nc.scalar.activation(
                            gt[:, ft, tci * TC:(tci + 1) * TC], ps,
                            AF.Gelu_apprx_tanh, bias=bias)
                last = fq == NFQ - 1
                for tt in range(ST2):
                    if last:
                        x1l2 = ln2p.tile([P, H], BF16, tag="x1l2")
                        nc.scalar.dma_start(x1l2,
                                            x1_dram[tt * P:(tt + 1) * P, :])
                        y_t = ln2p.tile([P, H], F32, tag="y2")
                    for hoc in range(NHOC):
                        o_sl = slice(hoc * HOC, (hoc + 1) * HOC)
                        ps2 = ops.tile([P, HOC], F32, tag="o")
                        for ft in range(FTQ):
                            nc.tensor.matmul(
                                ps2, gt[:, ft, tt * P:(tt + 1) * P],
                                w2q[:, ft, o_sl],
                                start=(ft == 0), stop=(ft == FTQ - 1))
                        if fq == 0:
                            nc.vector.tensor_copy(out2[:, tt, o_sl], ps2)
                        elif not last:
                            nc.vector.tensor_tensor(
                                out2[:, tt, o_sl], out2[:, tt, o_sl],
                                ps2, ALU.add)
                        else:
                            # fold the final quarter + LN2 residual per tile
                            nc.vector.tensor_tensor(
                                y_t[:, o_sl], out2[:, tt, o_sl], ps2, ALU.add)
                    if last:
                        nc.vector.tensor_tensor(y_t, y_t, x1l2, ALU.add)
                        if b2_sb is not None:
                            nc.vector.tensor_tensor(y_t, y_t, b2_sb, ALU.add)
                        o_t = ln2p.tile([P, H], F32, tag="o")
                        layernorm_tile(ln2p, y_t, o_t, ln2g_sb, ln2b_sb)
                        nc.sync.dma_start(out_ext[tt * P:(tt + 1) * P, :], o_t)


# ---------------------------------------------------------------------------
# host side
# ---------------------------------------------------------------------------

def _nonzero(a):
    return bool(np.any(np.asarray(a) != 0))


def compute_flags(inputs):
    flags = set()
    if _nonzero(inputs["attention_mask"]):
        flags.add("mask")
    for name in ["bq", "bk", "bv", "bo", "b1", "b2", "ln1_b", "ln2_b"]:
        if _nonzero(inputs[name]):
            flags.add(name)
    for name in ["ln1_g", "ln2_g"]:
        if bool(np.any(np.asarray(inputs[name]) != 1)):
            flags.add(name)
    return flags


def make_in_maps(S, H, FF, inputs, flags):
    """Shard full inputs into 8 per-core input maps (big tensors as bf16)."""
    import ml_dtypes
    bf16 = ml_dtypes.bfloat16
    SQ = S // 2
    x = np.asarray(inputs["x"], np.float32)       # [4, S, H]
    shared = {
        "wq": np.ascontiguousarray(np.asarray(inputs["Wq"], np.float32)).astype(bf16),
        "wk": np.ascontiguousarray(np.asarray(inputs["Wk"], np.float32)).astype(bf16),
        "wv": np.ascontiguousarray(np.asarray(inputs["Wv"], np.float32)).astype(bf16),
        "wo": np.ascontiguousarray(np.asarray(inputs["Wo"], np.float32)).astype(bf16),
        "w1": np.ascontiguousarray(np.asarray(inputs["W1"], np.float32)).astype(bf16),
        "w2": np.ascontiguousarray(np.asarray(inputs["W2"], np.float32)).astype(bf16),
    }
    for name in ["bq", "bk", "bv", "bo", "b1", "b2",
                 "ln1_g", "ln1_b", "ln2_g", "ln2_b"]:
        if name in flags:
            src = {"bq": "bq", "bk": "bk", "bv": "bv", "bo": "bo",
                   "b1": "b1", "b2": "b2", "ln1_g": "ln1_g", "ln1_b": "ln1_b",
                   "ln2_g": "ln2_g", "ln2_b": "ln2_b"}[name]
            shared[name] = np.ascontiguousarray(
                np.asarray(inputs[src], np.float32))
    xT_by_batch = [np.ascontiguousarray(x[b].T).astype(bf16) for b in range(4)]
    maps = []
    for c in range(8):
        b, j = divmod(c, 2)
        xTb = xT_by_batch[b]
        m = dict(shared)
        m["xT"] = xTb
        m["xqT"] = np.ascontiguousarray(xTb[:, j * SQ:(j + 1) * SQ])
        m["xh"] = np.ascontiguousarray(x[b, j * SQ:(j + 1) * SQ])
        if "mask" in flags:
            m["mask"] = np.ascontiguousarray(
                np.asarray(inputs["attention_mask"], np.float32)[b, 0, 0, :])
        maps.append(m)
    return maps


LAST_EXEC_NS = None
LAST_RESULTS = None


def _install_ntff_hook():
    """Register the NTFF profiling hook (missing antenv.axon_hooks shim)."""
    if "antenv.axon_hooks" in sys.modules:
        return
    try:
        import antenv  # noqa: F401
        mod = types.ModuleType("antenv.axon_hooks")
        hook = [None]
        mod.set_axon_ntff_profile_hook = lambda h: hook.__setitem__(0, h)
        mod.get_axon_ntff_profile_hook = lambda: hook[0]
        sys.modules["antenv.axon_hooks"] = mod
        from trn_agent_boot.trn_boot import _ntff_profile_via_ctypes
        mod.set_axon_ntff_profile_hook(
            _ntff_profile_via_ctypes("/opt/axon/libaxon_pjrt.so"))
    except Exception:
        sys.modules.pop("antenv.axon_hooks", None)


def run_block(S, H, FF, inputs, trace=False):
    """Build, compile, run on 8 cores; returns [B, S, H] output."""
    global LAST_EXEC_NS, LAST_RESULTS
    flags = compute_flags(inputs)
    nc = bacc.Bacc("TRN2", target_bir_lowering=False, debug=True)
    build_block(nc, S=S, H=H, NH_core=NH_CORE, FF=FF, flags=flags)
    nc.compile()
    in_maps = make_in_maps(S, H, FF, inputs, flags)
    if trace:
        _install_ntff_hook()
    res = run_bass_kernel_spmd(
        nc, in_maps, core_ids=list(range(8)), trace=trace,
        trace_cores=[0] if trace else None)
    LAST_EXEC_NS = res.exec_time_ns
    LAST_RESULTS = res
    SQ = S // 2
    B = 4
    out = np.empty((B, S, H), np.float32)
    for c in range(8):
        b, j = divmod(c, 2)
        out[b, j * SQ:(j + 1) * SQ] = res.results[c]["out"]
    return out


def kernel(x, attention_mask, Wq, bq, Wk, bk, Wv, bv, Wo, bo,
           ln1_g, ln1_b, W1, b1, W2, b2, ln2_g, ln2_b):
    inputs = dict(x=x, attention_mask=attention_mask, Wq=Wq, bq=bq, Wk=Wk,
                  bk=bk, Wv=Wv, bv=bv, Wo=Wo, bo=bo, ln1_g=ln1_g,
                  ln1_b=ln1_b, W1=W1, b1=b1, W2=W2, b2=b2, ln2_g=ln2_g,
                  ln2_b=ln2_b)
    trace = bool(int(os.environ.get("BLOCK_TRACE", "0")))
    return run_block(2048, 1024, 4096, inputs, trace=trace)


# revision 40
# speedup vs baseline: 1.0239x; 1.0142x over previous
"""Transformer block (post-LN, BERT-style) on 8 TRN2 NeuronCores, collective-free.

Sharding: 8 cores = 4 batches x 2 query-halves. Core c=(b,j) computes, for
batch b:
  - K/V projections for all 2048 tokens (recomputed per core pair; cheaper
    than any collective at this size),
  - Q projection + attention + output projection for its own 1024 query
    tokens (all 16 heads),
  - LN1, full FFN, LN2 for its 1024 tokens.
Host concatenates the 8 [1024, 1024] output slices. No collectives.

Layouts keep activations transposed ([feature, token]) so every matmul uses
weights in natural layout; x arrives pre-transposed from the host; softmax
row-sums come from a ones-column appended to V; 1/sqrt(hd) folds into the Q
projection epilogue.
"""

import os
import sys
import types
import numpy as np

import concourse.bacc as bacc
import concourse.bass as bass
import concourse.tile as tile
import concourse.mybir as mybir
from concourse.bass_utils import run_bass_kernel_spmd

P = 128
F32 = mybir.dt.float32
BF16 = mybir.dt.bfloat16
FP8 = mybir.dt.float8e4
AF = mybir.ActivationFunctionType
ALU = mybir.AluOpType

NH_CORE = 16  # heads per core (all of them; cores split over batch x seq)


def build_block(nc, *, S, H, NH_core, FF, eps=1e-12, flags=None, prefix=""):
    """Emit the SPMD program for one core. flags: set of optional-input names
    among {mask, bq, bk, bv, bo, b1, b2, ln1_g, ln1_b, ln2_g, ln2_b} that are
    actually present (nonzero / non-one)."""
    flags = flags or set()
    HD = 64
    NH = NH_core               # 16 heads, all on this core
    SQ = S // 2                # query tokens owned by this core
    HT = H // P                # 8 feature subtiles of H
    KT = S // P                # 16 k-token tiles
    KG = 2                     # k-tiles per exp batch (2 PSUM banks/slot)
    NKG = KT // KG
    QC = 512                   # query chunk (tokens per attention sweep)
    NQC = SQ // QC             # 2
    TC = 512                   # token chunk in projections
    HOC = 512                  # H-output chunk
    NHOC = H // HOC
    NFQ = 4                    # stream FFN weights in quarters
    FQ = FF // NFQ
    FTQ = FQ // P
    TT_Q = QC // P             # 4 token tiles per query chunk

    def pn(n):
        return f"{prefix}{n}"

    def param(name, shape, dt=F32):
        return nc.declare_dram_parameter(pn(name), list(shape), dt,
                                         isOutput=False)

    xT = param("xT", [H, S], BF16)
    xqT = param("xqT", [H, SQ], BF16)
    xh = param("xh", [SQ, H])
    wq = param("wq", [H, H], BF16)
    wk = param("wk", [H, H], BF16)
    wv = param("wv", [H, H], BF16)
    wo = param("wo", [H, H], BF16)
    w1 = param("w1", [H, FF], BF16)
    w2 = param("w2", [FF, H], BF16)
    opt = {}
    for name, shape in [("mask", [S]), ("bq", [H]), ("bk", [H]), ("bv", [H]),
                        ("bo", [H]), ("b1", [FF]), ("b2", [H]),
                        ("ln1_g", [H]), ("ln1_b", [H]),
                        ("ln2_g", [H]), ("ln2_b", [H])]:
        if name in flags:
            opt[name] = param(name, shape)
    out_ext = nc.declare_dram_parameter(pn("out"), [SQ, H], F32, isOutput=True)

    with (
        tile.TileContext(nc) as tc,
        tc.tile_pool(name=pn("singles"), bufs=1) as singles,
        tc.tile_pool(name=pn("dram"), bufs=1, space="DRAM") as dram,
    ):
        eps_sb = singles.tile([P, 1], F32)
        nc.vector.memset(eps_sb, eps)
        mask_sb = None
        if "mask" in flags:
            mask_sb = singles.tile([P, KT], F32)
            nc.gpsimd.dma_start(mask_sb, opt["mask"].rearrange("(a p) -> p a", p=P))

        # per-partition bias strips ([P, n//P]: feature f at [f%P, f//P])
        def col_strip(name, n):
            if name not in flags:
                return None
            t = singles.tile([P, n // P], F32, tag=f"strip_{name}")
            nc.gpsimd.dma_start(t, opt[name].rearrange("(a p) -> p a", p=P))
            return t
        bq_sb = col_strip("bq", H)
        bk_sb = col_strip("bk", H)
        b1_sb = col_strip("b1", FF)

        # partition-replicated rows (for free-dim adds)
        def rep_row(name, n):
            if name not in flags:
                return None
            t = singles.tile([P, n], F32, tag=f"rep_{name}")
            src = opt[name][:]
            bcast = bass.AP(tensor=src.tensor, offset=src.offset,
                            ap=[[0, P]] + list(src.ap))
            nc.gpsimd.dma_start(t, bcast)
            return t
        bv_sb = rep_row("bv", H)
        bo_sb = rep_row("bo", H)
        b2_sb = rep_row("b2", H)
        ln1g_sb = rep_row("ln1_g", H)
        ln1b_sb = rep_row("ln1_b", H)
        ln2g_sb = rep_row("ln2_g", H)
        ln2b_sb = rep_row("ln2_b", H)

        ones_sb = singles.tile([P, HD], F32)
        nc.vector.memset(ones_sb, 1.0)
        x1_dram = dram.tile([SQ, H], BF16)

        SG = 512                      # layernorm bn_stats chunk
        NSG = H // SG

        def layernorm_tile(lntp, y_t, out_sl, g_sb, b_sb):
            st6 = lntp.tile([P, NSG, 6], F32, tag="st6")
            for sg in range(NSG):
                nc.vector.bn_stats(st6[:, sg, :], y_t[:, sg * SG:(sg + 1) * SG])
            mv = lntp.tile([P, 2], F32, tag="mv")
            nc.vector.bn_aggr(mv, st6)
            nc.scalar.activation(mv[:, 1:2], mv[:, 1:2], AF.Sqrt, bias=eps_sb)
            nc.vector.reciprocal(mv[:, 1:2], mv[:, 1:2])
            nc.vector.tensor_scalar(out_sl, y_t, mv[:, 0:1], mv[:, 1:2],
                                    ALU.subtract, ALU.mult)
            if g_sb is not None:
                nc.vector.tensor_tensor(out_sl, out_sl, g_sb, ALU.mult)
            if b_sb is not None:
                nc.vector.tensor_tensor(out_sl, out_sl, b_sb, ALU.add)

        ST2 = SQ // P            # 8 token tiles
        x1T_c = [singles.tile([P, HT, QC], BF16, tag=f"x1T{i}",
                               name=f"x1T{i}")
                 for i in range(NQC)]
        with (
            tc.tile_pool(name=pn("wo"), bufs=1) as wop,
            tc.tile_pool(name=pn("stage"), bufs=3) as stagep,
            tc.tile_pool(name=pn("ctxk"), bufs=2) as ctxkp,
            tc.tile_pool(name=pn("ln1"), bufs=2) as ln1p,
            tc.tile_pool(name=pn("wo_ps"), bufs=1, space="PSUM") as wops,
        ):
            wo_sb = wop.tile([P, HT, H], BF16)
            nc.gpsimd.dma_start(wo_sb, wo.rearrange("(a p) h -> p a h", p=P))
            ctxT_per_qc = []
            keep_ctx = tc.tile_pool(name=pn("attn_keep"), bufs=1)
            keep = keep_ctx.__enter__()
            qT = keep.tile([P, HT, SQ], BF16)
            kT = keep.tile([P, HT, S], BF16)
            # fp8 V (+ones col): quantization error lands on the attention
            # output, which is self-normalized and tiny next to the residual.
            # Row stride HD+2 keeps every slice at an even byte offset; the
            # full-tile memset(1.0) provides the ones column contiguously.
            v_sb = keep.tile([P, KT, NH, HD + 2], FP8)
            nc.vector.memset(v_sb, 1.0)

            # ---------------- phase A: projections ------------------------
            with (
                tc.tile_pool(name=pn("qw"), bufs=1) as qwp,
                tc.tile_pool(name=pn("qkv_ps"), bufs=4, space="PSUM") as qps,
            ):
                # Q first: small DMA footprint, warms the PE early.
                xqT_sb = qwp.tile([P, HT, SQ], BF16)
                wq_sb = qwp.tile([P, HT, H], BF16)
                xqTr = xqT.rearrange("(a p) t -> p a t", p=P)
                nc.sync.dma_start(wq_sb, wq.rearrange("(a p) d -> p a d", p=P))
                for tci in range(SQ // TC):
                    t_sl = slice(tci * TC, (tci + 1) * TC)
                    nc.sync.dma_start(xqT_sb[:, :, t_sl], xqTr[:, :, t_sl])
                    for dt in range(HT):
                        ps = qps.tile([P, TC], F32, tag="qk")
                        for ht in range(HT):
                            nc.tensor.matmul(
                                ps, wq_sb[:, ht, dt * P:(dt + 1) * P],
                                xqT_sb[:, ht, t_sl],
                                start=(ht == 0), stop=(ht == HT - 1))
                        d_sl = qT[:, dt, t_sl]
                        if bq_sb is not None:
                            nc.vector.tensor_scalar(
                                d_sl, ps, bq_sb[:, dt:dt + 1], 0.125,
                                ALU.add, ALU.mult)
                        else:
                            nc.vector.tensor_scalar_mul(d_sl, ps, 0.125)

            with (
                tc.tile_pool(name=pn("kvw"), bufs=1) as kvwp,
                tc.tile_pool(name=pn("xtc"), bufs=2) as xtcp,
                tc.tile_pool(name=pn("kv_ps"), bufs=4, space="PSUM") as kvps,
                tc.tile_pool(name=pn("v_ps"), bufs=3, space="PSUM") as vps,
            ):
                wk_sb = kvwp.tile([P, HT, H], BF16)
                wv_sb = kvwp.tile([P, HT, H], BF16)
                # scalar-queue DMAs run in parallel with the sync-queue loads
                # above, so kT can start right after the Q matmuls. wv goes
                # after the first x chunk: V matmuls only start post-kT.
                nc.scalar.dma_start(wk_sb, wk.rearrange("(a p) d -> p a d", p=P))
                xTr = xT.rearrange("(a p) t -> p a t", p=P)
                for tci in range(S // TC):
                    t_sl = slice(tci * TC, (tci + 1) * TC)
                    xT_c = xtcp.tile([P, HT, TC], BF16, tag="xc")
                    nc.scalar.dma_start(xT_c, xTr[:, :, t_sl])
                    if tci == 0:
                        nc.scalar.dma_start(
                            wv_sb, wv.rearrange("(a p) d -> p a d", p=P))
                    for dt in range(HT):
                        ps = kvps.tile([P, TC], F32, tag="k")
                        for ht in range(HT):
                            nc.tensor.matmul(
                                ps, wk_sb[:, ht, dt * P:(dt + 1) * P],
                                xT_c[:, ht, :],
                                start=(ht == 0), stop=(ht == HT - 1))
                        d_sl = kT[:, dt, t_sl]
                        if bk_sb is not None:
                            nc.vector.tensor_scalar(
                                d_sl, ps, bk_sb[:, dt:dt + 1], 1.0,
                                ALU.add, ALU.mult)
                        else:
                            nc.vector.tensor_copy(d_sl, ps)
                    # V for this token chunk: token-major, ones col at HD
                    for lt in range(TC // P):
                        tt = tci * (TC // P) + lt
                        for dh in range(2):
                            ps = vps.tile([P, HOC], F32, tag="v")
                            for ht in range(HT):
                                nc.tensor.matmul(
                                    ps, xT_c[:, ht, lt * P:(lt + 1) * P],
                                    wv_sb[:, ht, dh * HOC:(dh + 1) * HOC],
                                    start=(ht == 0), stop=(ht == HT - 1))
                            if bv_sb is not None:
                                nc.vector.tensor_tensor(
                                    ps, ps, bv_sb[:, dh * HOC:(dh + 1) * HOC],
                                    ALU.add)
                            nc.vector.tensor_copy(
                                v_sb[:, tt, dh * 8:(dh + 1) * 8, 0:HD],
                                ps.rearrange("p (nh hd) -> p nh hd", hd=HD))

            # ---------------- phase B: attention -------------------------
            with (
                tc.tile_pool(name=pn("probs"), bufs=2) as probsp,
                tc.tile_pool(name=pn("sc_ps"), bufs=2, space="PSUM") as scp,
                tc.tile_pool(name=pn("ctx_ps"), bufs=3, space="PSUM") as ctxp,
            ):
                for qc in range(NQC):
                    q_sl = slice(qc * QC, (qc + 1) * QC)
                    ctxT = ctxkp.tile([P, HT, QC], BF16, tag=f"ctxT{qc}")
                    ctxT_per_qc.append(ctxT)
                    # Head-PAIR pipeline. Scores for heads (2hp, 2hp+1) sit at
                    # complementary partition halves, so their K=64 matmuls
                    # row-pack into the full PE array (HAM counts half-array
                    # matmuls as idle — packing keeps the clock at 2.4GHz).
                    # The previous pair's ctx matmuls interleave per k-tile so
                    # the PE never idles while ACT works through the exps.
                    prev = None
                    for hp in range(NH // 2 + 1):
                        if hp < NH // 2:
                            probs = probsp.tile([P, KT, 2, QC], FP8,
                                                tag="probs")
                        pce = pco = None
                        for kt in range(KT):
                            if hp < NH // 2:
                                ps_s = scp.tile([P, 2, QC], F32, tag="sc")
                                for par in range(2):
                                    hs = par * HD
                                    nc.tensor.matmul(
                                        ps_s[:, par, :],
                                        kT[hs:hs + HD, hp, kt * P:(kt + 1) * P],
                                        qT[hs:hs + HD, hp, q_sl],
                                        start=True, stop=True)
                                if mask_sb is not None:
                                    mvw = mask_sb[:, kt:kt + 1, None]
                                    nc.vector.tensor_tensor(
                                        ps_s, ps_s,
                                        mvw.to_broadcast((P, 2, QC)), ALU.add)
                                nc.scalar.activation(
                                    probs[:, kt, :, :], ps_s, AF.Exp)
                            if prev is not None:
                                php, pprobs = prev
                                if kt == 0:
                                    pce = ctxp.tile([P, QC], F32, tag="ctx")
                                    pco = ctxp.tile([P, QC], F32, tag="ctx")
                                nc.tensor.matmul(
                                    pce[0:HD + 1, :],
                                    v_sb[:, kt, 2 * php, 0:HD + 1],
                                    pprobs[:, kt, 0, :],
                                    start=(kt == 0), stop=(kt == KT - 1))
                                nc.tensor.matmul(
                                    pco[0:HD + 1, :],
                                    v_sb[:, kt, 2 * php + 1, 0:HD + 1],
                                    pprobs[:, kt, 1, :],
                                    start=(kt == 0), stop=(kt == KT - 1))
                        if prev is not None:
                            php, pprobs = prev
                            # softmax normalize: 1/rowsum broadcast over the
                            # 64 hd partitions via a K=1 fp32 matmul, fused
                            # into the PSUM->SBUF copy.
                            for par, ps_pc in ((0, pce), (1, pco)):
                                phs = par * HD
                                cs = stagep.tile([P, QC], BF16, tag="cs")
                                rr = stagep.tile([P, QC], F32, tag="rr")
                                nc.vector.reciprocal(
                                    rr[HD:HD + 1, :], ps_pc[HD:HD + 1, :])
                                rb = wops.tile([P, QC], F32, tag="rbwo")
                                nc.tensor.matmul(rb[0:HD, :],
                                                 ones_sb[HD:HD + 1, :],
                                                 rr[HD:HD + 1, :],
                                                 start=True, stop=True)
                                nc.vector.tensor_copy(cs[0:HD, :],
                                                      ps_pc[0:HD, :])
                                nc.vector.tensor_tensor(
                                    cs[0:HD, :], cs[0:HD, :],
                                    rb[0:HD, :], ALU.mult)
                                nc.sync.dma_start(ctxT[phs:phs + HD, php, :],
                                                  cs[0:HD, :])
                        if hp < NH // 2:
                            prev = (hp, probs)
                        else:
                            prev = None

                    # Wo + residual + LN1 for this query chunk (emitted here
                    # so the scheduler interleaves it with the next chunk's
                    # attention; the PE stream order is fixed at compile).
                    for tt in range(TT_Q):
                        tok0 = qc * QC + tt * P
                        xh_t = ln1p.tile([P, H], F32, tag="xh")
                        nc.sync.dma_start(xh_t, xh[tok0:tok0 + P, :])
                        y_t = ln1p.tile([P, H], F32, tag="y")
                        for hoc in range(NHOC):
                            o_sl = slice(hoc * HOC, (hoc + 1) * HOC)
                            ps_a = wops.tile([P, HOC], F32, tag="rbwo")
                            for st in range(HT):
                                nc.tensor.matmul(
                                    ps_a,
                                    ctxT[:, st, tt * P:(tt + 1) * P],
                                    wo_sb[:, st, o_sl],
                                    start=(st == 0), stop=(st == HT - 1))
                            nc.vector.tensor_tensor(y_t[:, o_sl], ps_a,
                                                    xh_t[:, o_sl], ALU.add)
                        if bo_sb is not None:
                            nc.vector.tensor_tensor(y_t, y_t, bo_sb, ALU.add)
                        x1b_t = ln1p.tile([P, H], BF16, tag="x1b")
                        layernorm_tile(ln1p, y_t, x1b_t, ln1g_sb, ln1b_sb)
                        nc.sync.dma_start(x1_dram[tok0:tok0 + P, :], x1b_t)
                        nc.sync.dma_start_transpose(
                            x1T_c[qc][:, :, tt * P:(tt + 1) * P], x1b_t)

            # close the K/Q/V pool before Wo+LN1 so the FFN's first tiles can
            # allocate (and its matmuls fill the transition gap).
            keep_ctx.__exit__(None, None, None)


        # ---------------- phase C: FFN + LN2 ------------------------------
        with (
            tc.tile_pool(name=pn("ffn_w"), bufs=2) as fwp,
            tc.tile_pool(name=pn("gt"), bufs=2) as gtp,
            tc.tile_pool(name=pn("out2"), bufs=1) as out2p,
            tc.tile_pool(name=pn("ln2"), bufs=2) as ln2p,
            tc.tile_pool(name=pn("h_ps"), bufs=4, space="PSUM") as hps,
            tc.tile_pool(name=pn("o_ps"), bufs=4, space="PSUM") as ops,
        ):
            out2 = out2p.tile([P, ST2, H], F32)
            for fq in range(NFQ):
                f_sl = slice(fq * FQ, (fq + 1) * FQ)
                # fq 0/1 on the (idle) gpsimd queue so the first FFN weights
                # land while attention still owns the sync queue.
                dma_eng = nc.gpsimd if fq < 2 else nc.sync
                w1q = fwp.tile([P, HT, FQ], BF16, tag="w1q")
                dma_eng.dma_start(
                    w1q, w1[:, f_sl].rearrange("(a p) f -> p a f", p=P))
                w2q = fwp.tile([P, FTQ, H], BF16, tag="w2q")
                dma_eng.dma_start(
                    w2q, w2[f_sl, :].rearrange("(a p) h -> p a h", p=P))
                gt = gtp.tile([P, FTQ, SQ], BF16, tag="gt")
                for ft in range(FTQ):
                    for tci in range(SQ // TC):
                        ps = hps.tile([P, TC], F32, tag="h")
                        for ht in range(HT):
                            nc.tensor.matmul(
                                ps, w1q[:, ht, ft * P:(ft + 1) * P],
                                x1T_c[tci][:, ht, :],
                                start=(ht == 0), stop=(ht == HT - 1))
                        bias = (b1_sb[:, fq * FTQ + ft:fq * FTQ + ft + 1]
                                if b1_sb is not None else 0.0)
                        nc.scalar.activation(
                            gt[:, ft, tci * TC:(tci + 1) * TC], ps,
                            AF.Gelu_apprx_tanh, bias=bias)
                last = fq == NFQ - 1
                for tt in range(ST2):
                    if last:
                        x1l2 = ln2p.tile([P, H], BF16, tag="x1l2")
                        nc.scalar.dma_start(x1l2,
                                            x1_dram[tt * P:(tt + 1) * P, :])
                        y_t = ln2p.tile([P, H], F32, tag="y2")
                    for hoc in range(NHOC):
                        o_sl = slice(hoc * HOC, (hoc + 1) * HOC)
                        ps2 = ops.tile([P, HOC], F32, tag="o")
                        for ft in range(FTQ):
                            nc.tensor.matmul(
                                ps2, gt[:, ft, tt * P:(tt + 1) * P],
                                w2q[:, ft, o_sl],
                                start=(ft == 0), stop=(ft == FTQ - 1))
                        if fq == 0:
                            nc.vector.tensor_copy(out2[:, tt, o_sl], ps2)
                        elif not last:
                            nc.vector.tensor_tensor(
                                out2[:, tt, o_sl], out2[:, tt, o_sl],
                                ps2, ALU.add)
                        else:
                            # fold the final quarter + LN2 residual per tile
                            nc.vector.tensor_tensor(
                                y_t[:, o_sl], out2[:, tt, o_sl], ps2, ALU.add)
                    if last:
                        nc.vector.tensor_tensor(y_t, y_t, x1l2, ALU.add)
                        if b2_sb is not None:
                            nc.vector.tensor_tensor(y_t, y_t, b2_sb, ALU.add)
                        o_t = ln2p.tile([P, H], F32, tag="o")
                        layernorm_tile(ln2p, y_t, o_t, ln2g_sb, ln2b_sb)
                        nc.sync.dma_start(out_ext[tt * P:(tt + 1) * P, :], o_t)


# ---------------------------------------------------------------------------
# host side
# ---------------------------------------------------------------------------

def _nonzero(a):
    return bool(np.any(np.asarray(a) != 0))


def compute_flags(inputs):
    flags = set()
    if _nonzero(inputs["attention_mask"]):
        flags.add("mask")
    for name in ["bq", "bk", "bv", "bo", "b1", "b2", "ln1_b", "ln2_b"]:
        if _nonzero(inputs[name]):
            flags.add(name)
    for name in ["ln1_g", "ln2_g"]:
        if bool(np.any(np.asarray(inputs[name]) != 1)):
            flags.add(name)
    return flags


def make_in_maps(S, H, FF, inputs, flags):
    """Shard full inputs into 8 per-core input maps (big tensors as bf16)."""
    import ml_dtypes
    bf16 = ml_dtypes.bfloat16
    SQ = S // 2
    x = np.asarray(inputs["x"], np.float32)       # [4, S, H]
    shared = {
        "wq": np.ascontiguousarray(np.asarray(inputs["Wq"], np.float32)).astype(bf16),
        "wk": np.ascontiguousarray(np.asarray(inputs["Wk"], np.float32)).astype(bf16),
        "wv": np.ascontiguousarray(np.asarray(inputs["Wv"], np.float32)).astype(bf16),
        "wo": np.ascontiguousarray(np.asarray(inputs["Wo"], np.float32)).astype(bf16),
        "w1": np.ascontiguousarray(np.asarray(inputs["W1"], np.float32)).astype(bf16),
        "w2": np.ascontiguousarray(np.asarray(inputs["W2"], np.float32)).astype(bf16),
    }
    for name in ["bq", "bk", "bv", "bo", "b1", "b2",
                 "ln1_g", "ln1_b", "ln2_g", "ln2_b"]:
        if name in flags:
            src = {"bq": "bq", "bk": "bk", "bv": "bv", "bo": "bo",
                   "b1": "b1", "b2": "b2", "ln1_g": "ln1_g", "ln1_b": "ln1_b",
                   "ln2_g": "ln2_g", "ln2_b": "ln2_b"}[name]
            shared[name] = np.ascontiguousarray(
                np.asarray(inputs[src], np.float32))
    xT_by_batch = [np.ascontiguousarray(x[b].T).astype(bf16) for b in range(4)]
    maps = []
    for c in range(8):
        b, j = divmod(c, 2)
        xTb = xT_by_batch[b]
        m = dict(shared)
        m["xT"] = xTb
        m["xqT"] = np.ascontiguousarray(xTb[:, j * SQ:(j + 1) * SQ])
        m["xh"] = np.ascontiguousarray(x[b, j * SQ:(j + 1) * SQ])
        if "mask" in flags:
            m["mask"] = np.ascontiguousarray(
                np.asarray(inputs["attention_mask"], np.float32)[b, 0, 0, :])
        maps.append(m)
    return maps


LAST_EXEC_NS = None
LAST_RESULTS = None


def _install_ntff_hook():
    """Register the NTFF profiling hook (missing antenv.axon_hooks shim)."""
    if "antenv.axon_hooks" in sys.modules:
        return
    try:
        import antenv  # noqa: F401
        mod = types.ModuleType("antenv.axon_hooks")
        hook = [None]
        mod.set_axon_ntff_profile_hook = lambda h: hook.__setitem__(0, h)
        mod.get_axon_ntff_profile_hook = lambda: hook[0]
        sys.modules["antenv.axon_hooks"] = mod
        from trn_agent_boot.trn_boot import _ntff_profile_via_ctypes
        mod.set_axon_ntff_profile_hook(
            _ntff_profile_via_ctypes("/opt/axon/libaxon_pjrt.so"))
    except Exception:
        sys.modules.pop("antenv.axon_hooks", None)


def run_block(S, H, FF, inputs, trace=False):
    """Build, compile, run on 8 cores; returns [B, S, H] output."""
    global LAST_EXEC_NS, LAST_RESULTS
    flags = compute_flags(inputs)
    nc = bacc.Bacc("TRN2", target_bir_lowering=False, debug=True)
    build_block(nc, S=S, H=H, NH_core=NH_CORE, FF=FF, flags=flags)
    nc.compile()
    in_maps = make_in_maps(S, H, FF, inputs, flags)
    if trace:
        _install_ntff_hook()
    res = run_bass_kernel_spmd(
        nc, in_maps, core_ids=list(range(8)), trace=trace,
        trace_cores=[0] if trace else None)
    LAST_EXEC_NS = res.exec_time_ns
    LAST_RESULTS = res
    SQ = S // 2
    B = 4
    out = np.empty((B, S, H), np.float32)
    for c in range(8):
        b, j = divmod(c, 2)
        out[b, j * SQ:(j + 1) * SQ] = res.results[c]["out"]
    return out


def kernel(x, attention_mask, Wq, bq, Wk, bk, Wv, bv, Wo, bo,
           ln1_g, ln1_b, W1, b1, W2, b2, ln2_g, ln2_b):
    inputs = dict(x=x, attention_mask=attention_mask, Wq=Wq, bq=bq, Wk=Wk,
                  bk=bk, Wv=Wv, bv=bv, Wo=Wo, bo=bo, ln1_g=ln1_g,
                  ln1_b=ln1_b, W1=W1, b1=b1, W2=W2, b2=b2, ln2_g=ln2_g,
                  ln2_b=ln2_b)
    trace = bool(int(os.environ.get("BLOCK_TRACE", "0")))
    return run_block(2048, 1024, 4096, inputs, trace=trace)
